# revision 27
# baseline (speedup 1.0000x reference)
"""Trainium2 Bass kernel for nn_CrossModalAttention (sparse per-channel 3x3
token-window attention).

Contract: kernel(**inputs) takes the FULL fp32 inputs (B=8,C=256,H=W=64) and
returns the FULL fp32 output.  Internally: data-parallel over batch across the
8 NeuronCores (1 batch element per core), params replicated.

Layout: everything on-chip is d-major — activations are [c, d, t] where d is
the pixel-within-token index (16) and t = (I,J) the token index (256).  This
makes every DVE elementwise op innermost-contiguous over t (bf16 2x mode),
makes the per-d / per-n PE reduction matmuls read contiguous columns, and
turns the attention-weight broadcast over d into a free outer-dim 0-stride.

Per-core pipeline:
  PE  : Q/K/V 1x1-conv projections (bf16, contraction over input channels);
        QK d-reduction and AV 9-neighbor accumulation as identity-weight
        accumulating matmuls into PSUM (fp32).
  DVE : QK and AV elementwise products (bf16 2x), softmax sum/recip/scale,
        final residual add.
  ACT : PSUM evacuations with fused bias+cast, exp().
  DMA : fully contiguous loads (host pre-permutes), store.

dj=+1 neighbor views start at odd element offsets, so those 3 of 9 products
run at DVE 1x instead of 2x (inherent to an odd token shift on a stride-1 J).
"""

import os
import sys
from contextlib import ExitStack

import numpy as np

for _p in ("/opt/trn_rl_repo",):
    if _p not in sys.path and os.path.isdir(_p):
        sys.path.insert(0, _p)

import ml_dtypes  # noqa: E402

import concourse.bacc as bacc  # noqa: E402
import concourse.bass as bass  # noqa: E402
import concourse.tile as tile  # noqa: E402
from concourse import mybir  # noqa: E402
from concourse.bass_utils import run_bass_kernel_spmd  # noqa: E402

BF16 = mybir.dt.bfloat16
F32 = mybir.dt.float32
ALU = mybir.AluOpType
ACTF = mybir.ActivationFunctionType

B, C, H, W = 8, 256, 64, 64
TS = 4                      # token size
NH = H // TS                # 16 token rows
NW = W // TS                # 16 token cols
T = NH * NW                 # 256 tokens
D = TS * TS                 # 16 pixels per token
G = 2                       # channel groups of 128
P = 128
PIX = H * W                 # 4096
GRID = NH + 2               # 18 (zero-padded token grid)
SCALE = float(D) ** -0.5    # 0.25
N_CORES = 8

_BUILT = None


def _emit(ctx: ExitStack, tc: "tile.TileContext"):
    nc = tc.nc

    # ---- DRAM I/O (per-core shard); activations d-major [p, g, d, t] ----
    xb_d = nc.dram_tensor("xb", [P, G, PIX], BF16, kind="ExternalInput").ap()
    xw_d = nc.dram_tensor("xw", [P, G, PIX], BF16, kind="ExternalInput").ap()
    sm_d = nc.dram_tensor("smalls", [P, 1670], BF16, kind="ExternalInput").ap()
    out_d = nc.dram_tensor("out", [P, G, PIX], F32, kind="ExternalOutput").ap()

    consts = ctx.enter_context(tc.tile_pool(name="consts", bufs=1))
    prod = ctx.enter_context(tc.tile_pool(name="prod", bufs=3))
    avp = ctx.enter_context(tc.tile_pool(name="avp", bufs=3))
    outp = ctx.enter_context(tc.tile_pool(name="outf", bufs=2))

    # ---- persistent SBUF tiles ----
    xb = consts.tile([P, G, PIX], BF16)          # blue, d-major token order
    sm = consts.tile([P, 1670], BF16)            # packed: ident | W^T | biases
    ident = sm[:, 0:P]
    wall = sm[:, P:P + 3 * G * C].rearrange("p (w g c) -> p w g c", w=3, g=G)
    ball = sm[:, P + 3 * G * C:]                 # bf16 biases, col = proj*2+g
    qsb = consts.tile([P, G, D, T], BF16)
    kvp = consts.tile([P, 2, G, D, GRID, GRID], BF16)  # padded K (0) / V (1)
    kun = consts.tile([P, G, D, T], BF16)              # K unpadded (dj=1 views)
    esb = consts.tile([P, G, 10, T], BF16)       # exp(logits), slot-major
    sr = consts.tile([P, 2, G, T], F32)          # softmax sum / reciprocal

    nc.sync.dma_start(xb[:], xb_d[:])
    nc.sync.dma_start(sm[:], sm_d[:])

    # zero the padding ring of the K/V token grids (rows and cols 0,17)
    for kv in range(2):
        for g in range(G):
            nc.gpsimd.memset(kvp[:, kv, g, :, 0:GRID:GRID - 1, :], 0.0)
            nc.gpsimd.memset(kvp[:, kv, g, :, :, 0:GRID:GRID - 1], 0.0)

    # ---- projections: out[c, d, t] = sum_a W[c,a] x[a, d, t] + b[c] ----
    CH = 1024  # psum chunk: 4 d-planes x 256 t (2 banks)
    with tc.tile_pool(name="psumP", bufs=2, space="PSUM") as psum:
        with tc.tile_pool(name="xwp", bufs=1) as xwpool:
            xw = xwpool.tile([P, G, PIX], BF16, tag="xw")
            nc.sync.dma_start(xw[:], xw_d[:])
            for proj, src in ((0, xb), (1, xw), (2, xw)):
                for g in range(G):
                    bias_ap = ball[:, proj * 2 + g: proj * 2 + g + 1]
                    for u in range(PIX // CH):  # 4 chunks of 4 d-planes
                        pt = psum.tile([P, CH], F32)
                        for h in range(2):  # h-outer: one weight load per half
                            for j in range(CH // 512):
                                cols = slice(u * CH + j * 512, u * CH + (j + 1) * 512)
                                mm = nc.tensor.matmul(
                                    pt[:, j * 512:(j + 1) * 512],
                                    wall[:, proj, h, g * P:(g + 1) * P],
                                    src[:, h, cols],
                                    start=(h == 0),
                                    stop=(h == 1),
                                )
                                if j > 0:
                                    mm.ins.ldweights = False
                        pv = pt[:].rearrange("p (d i j) -> p d i j", d=4, i=NH)
                        ds = slice(4 * u, 4 * u + 4)
                        if proj == 0:  # Q flat d-major
                            nc.scalar.activation(
                                qsb[:, g, ds, :], pt[:], ACTF.Identity, bias=bias_ap
                            )
                        else:  # K/V -> padded grid interior (fused bias+cast)
                            nc.scalar.activation(
                                kvp[:, proj - 1, g, ds, 1:1 + NH, 1:1 + NW],
                                pv, ACTF.Identity, bias=bias_ap,
                            )
                            if proj == 1:  # K also unpadded (aligned dj=1 views)
                                nc.scalar.activation(
                                    kun[:, g, ds, :], pt[:], ACTF.Identity,
                                    bias=bias_ap,
                                )

    # ---- attention ----
    # neighbor n=(di,dj), di,dj in {0,1,2}: view base (di,dj) in the padded
    # grid.  dj=1 views start at odd offsets -> those products run 1x.
    def kview(g, di, dj):
        return kvp[:, 0, g, :, di:di + NH, dj:dj + NW]

    PAIRS = ((0, 1), (2, 3), (4, 5), (6, 7), (8,))
    psL = ctx.enter_context(tc.tile_pool(name="psumL", bufs=2, space="PSUM"))
    psA = ctx.enter_context(tc.tile_pool(name="psumA", bufs=1, space="PSUM"))
    for g in range(G):
        qv = qsb[:, g]  # [P, D, T]
        # logits l[c,t,n] = sum_d q[c,d,t] k_n[c,d,t]  (scale folded into exp)
        # products computed in d-halves so the PE reduction starts early
        for pair in PAIRS:
            halves = []
            for dh in range(2):
                ph = prod.tile([P, 2, D // 2, T], BF16, tag="prod")
                dsl = slice(8 * dh, 8 * dh + 8)
                for w, n in enumerate(pair):
                    di, dj = n // 3, n % 3
                    if dj != 1:
                        nc.vector.tensor_tensor(
                            ph[:, w], qv[:, dsl, :],
                            kview(g, di, dj)[:, dsl, :, :], op=ALU.mult,
                        )
                    elif di == 1:  # center: unpadded K, fully aligned
                        nc.vector.tensor_tensor(
                            ph[:, w], qv[:, dsl, :], kun[:, g, dsl, :], op=ALU.mult,
                        )
                    elif di == 0:  # K row I-1: valid for t >= 16; zero-fill row 0
                        nc.gpsimd.memset(ph[:, w, :, 0:16], 0.0)
                        nc.vector.tensor_tensor(
                            ph[:, w, :, 16:T], qv[:, dsl, 16:T],
                            kun[:, g, dsl, 0:T - 16], op=ALU.mult,
                        )
                    else:  # di == 2: K row I+1: valid for t < 240; zero row 15
                        nc.gpsimd.memset(ph[:, w, :, T - 16:T], 0.0)
                        nc.vector.tensor_tensor(
                            ph[:, w, :, 0:T - 16], qv[:, dsl, 0:T - 16],
                            kun[:, g, dsl, 16:T], op=ALU.mult,
                        )
                halves.append(ph)
            lp = psL.tile([P, 512], F32, tag="psL")
            for d in range(D):  # accumulate over d on PE (identity weights)
                mm = nc.tensor.matmul(
                    lp[:, :len(pair) * T],
                    ident[:],
                    halves[d // 8][:, 0:len(pair), d % 8, :],
                    start=(d == 0),
                    stop=(d == D - 1),
                )
                if d > 0:  # identity already resident (group leader loaded it)
                    mm.ins.ldweights = False
            nc.scalar.activation(  # e = exp(scale*l) -> [slot, t]
                esb[:, g, pair[0]:pair[0] + len(pair), :],
                lp[:, :len(pair) * T], ACTF.Exp, scale=SCALE,
            )
        # softmax denominator via pairwise tree (fp32); then e <- e * (1/s)
        s4 = sr[:, 0, g, :]
        t9 = outp.tile([P, 4, T], F32, tag="sden")
        for k in range(4):
            nc.vector.tensor_tensor(
                t9[:, k], esb[:, g, 2 * k, :], esb[:, g, 2 * k + 1, :], op=ALU.add
            )
        nc.vector.tensor_tensor(t9[:, 0], t9[:, 0], t9[:, 1], op=ALU.add)
        nc.vector.tensor_tensor(t9[:, 2], t9[:, 2], t9[:, 3], op=ALU.add)
        nc.vector.tensor_tensor(t9[:, 0], t9[:, 0], t9[:, 2], op=ALU.add)
        nc.vector.tensor_tensor(s4, t9[:, 0], esb[:, g, 8, :], op=ALU.add)
        nc.vector.reciprocal(sr[:, 1, g, :], s4)
        ev = esb[:, g, 0:9, :]
        nc.vector.tensor_tensor(
            ev, ev,
            sr[:, 1, g, :].unsqueeze(1).broadcast_to([P, 9, T]),
            op=ALU.mult,
        )

        # enhanced[c,d,t] = sum_n p_n[c,t] * v_n[c,d,t], accumulated on PE,
        # in d-halves of 8 planes (4 PSUM banks each)
        for hf in range(2):
            dsl = slice(8 * hf, 8 * hf + 8)
            acc = psA.tile([P, 8 * T], F32, tag="psA")
            for n in range(9):
                di, dj = n // 3, n % 3
                if dj == 1:
                    vv = kvp[:, 1, g, dsl, di:di + NH, 1:1 + NW]  # 1x (odd)
                else:
                    vv = kvp[:, 1, g, dsl, di:di + NH, dj:dj + NW]
                pe = esb[:, g, n, :].unsqueeze(1).broadcast_to([P, 8, T])
                tn = avp.tile([P, 8, T], BF16, tag="avprod")
                nc.vector.tensor_tensor(tn[:], vv, pe, op=ALU.mult)
                tf = tn[:].rearrange("p d t -> p (d t)")
                for j in range(4):
                    mm = nc.tensor.matmul(
                        acc[:, j * 512:(j + 1) * 512],
                        ident[:],
                        tf[:, j * 512:(j + 1) * 512],
                        start=(n == 0),
                        stop=(n == 8),
                    )
                    if not (n == 0 and j == 0):
                        mm.ins.ldweights = False
            # out = enhanced + blue
            of = outp.tile([P, 8 * T], F32, tag="outf")
            nc.vector.tensor_tensor(
                of[:], acc[:], xb[:, g, 8 * hf * T:(8 * hf + 8) * T], op=ALU.add
            )
            nc.sync.dma_start(out_d[:, g, 8 * hf * T:(8 * hf + 8) * T], of[:])


def _build():
    global _BUILT
    if _BUILT is None:
        nc = bacc.Bacc(
            "TRN2", target_bir_lowering=False, debug=False, num_devices=N_CORES
        )
        with tile.TileContext(nc) as tc:
            with ExitStack() as ctx:
                _emit(ctx, tc)
        nc.compile()
        _BUILT = nc
    return _BUILT


def _tokenize(x: np.ndarray) -> np.ndarray:
    """[C,H,W] -> [C, D*T] d-major token order: index = (u,v,I,J)."""
    c = x.shape[0]
    return (
        x.reshape(c, NH, TS, NW, TS).transpose(0, 2, 4, 1, 3).reshape(c, PIX)
    )


def _untokenize(y: np.ndarray) -> np.ndarray:
    """[C, D*T] d-major token order -> [C, H, W]."""
    c = y.shape[0]
    return (
        y.reshape(c, TS, TS, NH, NW).transpose(0, 3, 1, 4, 2).reshape(c, H, W)
    )


def _part_fold(x: np.ndarray) -> np.ndarray:
    """[C, F] -> [P, C//P, F] partition-major fold."""
    return np.ascontiguousarray(
        x.reshape(C // P, P, -1).transpose(1, 0, 2)
    )


def _prep_maps(blue_feat, white_feat, Wq, bq, Wk, bk, Wv, bv):
    bf16 = ml_dtypes.bfloat16
    wall = np.stack([np.asarray(w, np.float32).T for w in (Wq, Wk, Wv)])  # [3,a,c]
    wall = np.ascontiguousarray(
        wall.reshape(3, 2, P, C).transpose(2, 0, 1, 3)
    ).reshape(P, 3 * G * C)  # [P, (proj, a_hi, c)]
    ball = np.ascontiguousarray(
        np.stack([bq, bk, bv]).astype(np.float32).reshape(3, G, P).transpose(2, 0, 1)
    ).reshape(P, 6)
    smalls = np.concatenate(
        [np.eye(P, dtype=np.float32), wall, ball], axis=1
    ).astype(bf16)  # [P, 1670]: ident | W^T | biases
    maps = []
    for b in range(B):
        xbm = _part_fold(_tokenize(np.asarray(blue_feat[b], np.float32))).astype(bf16)
        xwm = _part_fold(_tokenize(np.asarray(white_feat[b], np.float32))).astype(bf16)
        maps.append({"xb": xbm, "xw": xwm, "smalls": smalls})
    return maps


def _gather(results) -> np.ndarray:
    out = np.empty((B, C, H, W), np.float32)
    for b in range(B):
        y = results[b]["out"]  # [P, G, PIX] f32
        y = np.asarray(y, np.float32).transpose(1, 0, 2).reshape(C, PIX)
        out[b] = _untokenize(y)
    return out


def _install_ntff_hook():
    """The agent image's antenv lacks axon_hooks; synthesize it so
    run_bass_kernel_spmd(trace=True) can drive NTFF profiling via the
    injected libaxon_pjrt.so C ABI (mirrors trn_agent_boot.trn_boot)."""
    import contextlib
    import ctypes
    import types

    if "antenv.axon_hooks" in sys.modules:
        return
    so_path = "/opt/axon/libaxon_pjrt.so"
    lib = ctypes.CDLL(so_path)
    if not hasattr(lib, "axon_start_nrt_profile"):
        return
    lib.axon_start_nrt_profile.argtypes = [
        ctypes.POINTER(ctypes.c_int64),
        ctypes.c_size_t,
    ]
    lib.axon_start_nrt_profile.restype = ctypes.c_int64
    lib.axon_stop_nrt_profile.argtypes = [ctypes.c_char_p]
    lib.axon_stop_nrt_profile.restype = ctypes.c_int64

    @contextlib.contextmanager
    def _hook(output_dir, device_ids):
        import jax

        jax.devices()
        if device_ids:
            ids = (ctypes.c_int64 * len(device_ids))(*device_ids)
            rc = lib.axon_start_nrt_profile(ids, len(device_ids))
        else:
            rc = lib.axon_start_nrt_profile(None, 0)
        if rc != 0:
            raise RuntimeError(f"axon_start_nrt_profile rc={rc}")
        try:
            yield
        finally:
            n = lib.axon_stop_nrt_profile(str(output_dir).encode())
            print(f"ntff profile: {n} file(s) written to {output_dir}")

    mod = types.ModuleType("antenv.axon_hooks")
    mod.get_axon_ntff_profile_hook = lambda: _hook  # type: ignore[attr-defined]
    mod.set_axon_ntff_profile_hook = lambda h: None  # type: ignore[attr-defined]
    sys.modules["antenv.axon_hooks"] = mod


def run(trace=False, **inputs):
    nc = _build()
    maps = _prep_maps(**inputs)
    if trace:
        _install_ntff_hook()
    res = run_bass_kernel_spmd(nc, maps, list(range(N_CORES)), trace=trace)
    return _gather(res.results), res


def kernel(**inputs) -> np.ndarray:
    out, _ = run(trace=False, **inputs)
    return out
